# revision 42
# baseline (speedup 1.0000x reference)
"""nn_CPN_67740224192953 kernel: detection pipeline on 8 trn2 cores.

Device (8 cores, 2 per image = half-image each; bottom halves are
processed vertically flipped with row-flipped weights so the zero-pad
boundary is uniform across cores), fp32 compute, ~2ms NEFF:
  - im2col of the 3-channel x canvas via 9 strided DMAs (27-row rhs)
  - backbone 3x3 conv as K=27 matmuls + relu(+bias) on ACT
    -> f [64, 70*518] SBUF per 64-row superslab
  - head convs for the 3 channels needing full-res maps
    (d = score1-score0, ref_x, ref_y) factored over dy:
      stage1: 7 accumulating matmuls (K=64, M=21=(c,dx)) into PSUM
      stage2: PSUM->SBUF copy, reshape-DMA into a (c*32+jj)-partition
      block tile Q''[.., dx*512+u], one shear tensor_reduce over dx
  - ref channels dumped bf16; the d plane never leaves the device:
    per-partition top-32 candidates (max_with_indices/match_replace,
    4 rounds) + a first-dropped certificate value are shipped instead
    (~0.57MB/core out, ~1.7MB/core in through the axon tunnel)
The device program and a jitted SPMD runner (device-resident zero
output operands, no per-call retrace) are cached at module level.
Host: bias/tanh/margin, exact top-k from the shipped candidates (a
strict-dominance certificate guards exactness; host conv fallback if
it ever fails), loc/fourier head at the 512 detections (recomputed
from x patches), fourier contour synthesis, 4 refinement gather
iterations (mirrors reference).
"""

import numpy as np

LAST_EXEC_NS = None
LAST_DEVICE_S = None

B, C_IN, H, W = 4, 3, 512, 512
C = 64
ORDER = 5
SAMPLES = 32
N_DET = 512
ITERS = 4
MARGIN = 3.0
K7 = 7

HALF = 256            # output rows per core
WP = 518              # f-space width (x_img = q-3, q in [0,518))
CANH, CANW = 264, 520 # x canvas per core (y_img = i-4+256h, x_img = j-4)
SS = 64               # output rows per superslab
NSS = HALF // SS      # 4 superslabs
FR = SS + 6           # f rows per superslab
BROWS = 7             # f rows per backbone band
NBAND = (FR + BROWS - 1) // BROWS   # 10 bands
BPOS = BROWS * WP     # f positions per full band
NPOS = SS * WP        # 33152 out positions per superslab
CH = 506              # out positions per head chunk (512 - 6 halo)
NCH = (NPOS + CH - 1) // CH   # 66 chunks (65*506 + 262)
BLKS = (32, 32, 2)    # chunks per stage2 block (c*32+jj, quadrant-aligned)
FALLOC = FR * WP + 252        # f slack: last chunk reads to 36510
PLANE = NCH * CH              # 33396 positions per superslab incl. tail pad
NTOPR = 32            # top-R values extracted per partition (4 rounds of 8)
DALLW = 3 * CH        # d_all free width (1518; 264 chunk rows in 128x3 slots)
TOPW = 128 * NTOPR * 2 + 128 * 8      # u32 words: values + indices + dropped
XCN = C_IN * CANH * CANW      # canvas floats
XINLEN = XCN + 27 * C + C * 147 + C   # packed input floats per core

_CACHED_NC = None
_RUNNER = None


def _make_runner(nc):
    """Jitted SPMD runner built once and reused across kernel() calls
    (mirrors bass2jax.run_bass_via_pjrt's multi-core path; caching the
    traced executable removes per-call retrace/lowering overhead)."""
    import jax
    import numpy as np
    from jax.experimental.shard_map import shard_map
    from jax.sharding import Mesh, PartitionSpec
    from concourse import bass2jax, mybir

    bass2jax.install_neuronx_cc_hook()
    n_cores = 8
    partition_name = (nc.partition_id_tensor.name
                      if nc.partition_id_tensor else None)
    in_names, out_names, out_avals = [], [], []
    for alloc in nc.m.functions[0].allocations:
        if not isinstance(alloc, mybir.MemoryLocationSet):
            continue
        name = alloc.memorylocations[0].name
        if alloc.kind == "ExternalInput":
            if name != partition_name:
                in_names.append(name)
        elif alloc.kind == "ExternalOutput":
            shape = tuple(alloc.tensor_shape)
            dtype = mybir.dt.np(alloc.dtype)
            out_names.append(name)
            out_avals.append(jax.core.ShapedArray(shape, dtype))
    n_params = len(in_names)
    all_names = in_names + out_names
    if partition_name is not None:
        all_names = all_names + [partition_name]
    donate = tuple(range(n_params, n_params + len(out_names)))

    def _body(*args):
        operands = list(args)
        if partition_name is not None:
            operands.append(bass2jax.partition_id_tensor())
        outs = bass2jax._bass_exec_p.bind(
            *operands,
            out_avals=tuple(out_avals),
            in_names=tuple(all_names),
            out_names=tuple(out_names),
            lowering_input_output_aliases=(),
            sim_require_finite=True,
            sim_require_nnan=True,
            nc=nc,
        )
        return tuple(outs)

    devices = jax.devices()[:n_cores]
    mesh = Mesh(np.asarray(devices), ("core",))
    nargs = n_params + len(out_names)
    sharded = jax.jit(
        shard_map(_body, mesh=mesh,
                  in_specs=(PartitionSpec("core"),) * nargs,
                  out_specs=(PartitionSpec("core"),) * len(out_names),
                  check_rep=False),
        keep_unused=True)

    # outputs are fully written by the NEFF, so the zero "output operand"
    # buffers never need refreshing: keep them device-resident across calls
    from jax.sharding import NamedSharding
    shard8 = NamedSharding(mesh, PartitionSpec("core"))
    zeros_dev = [jax.device_put(
        np.zeros((n_cores * a.shape[0], *a.shape[1:]), a.dtype), shard8)
        for a in out_avals]

    def run(in_maps):
        per_core = [[np.asarray(m[name]) for name in in_names] for m in in_maps]
        concat_in = [np.concatenate([per_core[c][i] for c in range(n_cores)], 0)
                     for i in range(n_params)]
        outs = sharded(*concat_in, *zeros_dev)
        return [{name: np.asarray(outs[i]).reshape(n_cores, *out_avals[i].shape)[c]
                 for i, name in enumerate(out_names)}
                for c in range(n_cores)]

    return run


def _build_device_program(num_devices=8, nss=NSS):
    import concourse.bacc as bacc
    import concourse.mybir as mybir

    nc = bacc.Bacc("TRN2", target_bir_lowering=False, num_devices=num_devices)
    f32 = mybir.dt.float32
    xin_d = nc.dram_tensor("xin", [XINLEN], f32, kind="ExternalInput")
    xc_d = xin_d[0:XCN].rearrange("(c h w) -> c h w", c=C_IN, h=CANH)
    w27_d = xin_d[XCN:XCN + 27 * C].rearrange("(a b) -> a b", a=27)
    wh_d = xin_d[XCN + 27 * C:XCN + 27 * C + C * 147].rearrange(
        "(a b) -> a b", a=C)
    bbb_d = xin_d[XINLEN - C:XINLEN].rearrange("(a b) -> a b", a=C)
    out_d = nc.dram_tensor("out", [nss * PLANE + TOPW], mybir.dt.uint32,
                           kind="ExternalOutput")
    _device_body(nc, xc_d, w27_d, wh_d, bbb_d, out_d, nss)
    nc.finalize()
    return nc


def _device_body(nc, xc_d, w27_d, wh_d, bbb_d, out_d, nss=NSS):
    import concourse.mybir as mybir
    from concourse.tile import TileContext

    f32 = mybir.dt.float32
    with (
        TileContext(nc) as tc,
        tc.tile_pool(name="wpool", bufs=1) as wpool,
        tc.tile_pool(name="sbi", bufs=2) as sbi,
        tc.tile_pool(name="sbf", bufs=1) as sbf,
        tc.tile_pool(name="sbq", bufs=1) as sbq,
        tc.tile_pool(name="sbs", bufs=1) as sbs,
        tc.tile_pool(name="sbr", bufs=1) as sbr,
        tc.tile_pool(name="sbb", bufs=1) as sbb,
        tc.tile_pool(name="sbk", bufs=1) as sbk,
        tc.tile_pool(name="ps", bufs=2, space="PSUM") as ps,
        tc.tile_pool(name="psq", bufs=4, space="PSUM") as psq,
    ):
        # weights: DMA into one packed staging tile, DVE-copy to a packed
        # working tile (cheap matmul weight deps); layout: cols [0:64) w27
        # (parts 0:27), [64:211) wh (parts 0:64), [211:212) bbb
        wst = wpool.tile([C, 212], f32, tag="wst")
        nc.sync.dma_start(out=wst[0:27, 0:C], in_=w27_d[:, :])
        nc.sync.dma_start(out=wst[0:C, C:C + 147], in_=wh_d[:, :])
        nc.sync.dma_start(out=wst[0:C, 211:212], in_=bbb_d[:, :])
        wpk = wpool.tile([C, 212], f32, tag="wpk")
        nc.vector.tensor_copy(wpk[0:27, 0:C], wst[0:27, 0:C])
        nc.vector.tensor_copy(wpk[0:C, C:C + 147], wst[0:C, C:C + 147])
        nc.vector.tensor_copy(wpk[0:C, 211:212], wst[0:C, 211:212])
        w27_t = wpk[0:27, 0:C]
        wh_t = wpk[0:C, C:C + 147]
        bbb_t = wpk[0:C, 211:212]

        dall = sbk.tile([128, DALLW], f32, tag="dall")
        nc.vector.memset(dall[:], -1e30)

        for s in range(nss):
            f_t = sbf.tile([C, FALLOC], f32, tag="f")
            # ---- backbone: f = relu(w27.T @ im2col(x) + b) ----
            for band in range(NBAND):
                brows = min(BROWS, FR - band * BROWS)
                bpos = brows * WP
                r0 = s * SS + band * BROWS   # global f row of band start
                imc = sbi.tile([27, BPOS], f32, tag="imc")
                for dy2 in range(3):
                    for dx2 in range(3):
                        k = dy2 * 3 + dx2
                        nc.sync.dma_start(
                            out=imc[3 * k:3 * k + 3, :bpos],
                            in_=xc_d[:, r0 + dy2:r0 + dy2 + brows,
                                     dx2:dx2 + WP])
                boff = band * BPOS
                for c0 in range(0, bpos, 512):
                    n = min(512, bpos - c0)
                    pbb = ps.tile([C, 512], f32, tag="pbb")
                    nc.tensor.matmul(out=pbb[:, :n], lhsT=w27_t,
                                     rhs=imc[:, c0:c0 + n],
                                     start=True, stop=True)
                    nc.scalar.activation(
                        f_t[:, boff + c0:boff + c0 + n], pbb[:, :n],
                        mybir.ActivationFunctionType.Relu,
                        bias=bbb_t)
            # f must be zero where the head conv expects zero padding:
            # cols q<3 / q>=515 (x_img outside) on every superslab, and
            # rows rf<3 of superslab 0 (y outside; bottom cores get a
            # vertically flipped canvas so the edge is always local-top).
            f3d = f_t[:, 0:FR * WP].rearrange("p (r q) -> p r q", q=WP)
            nc.vector.memset(f3d[:, :, 0:3], 0.0)
            nc.vector.memset(f3d[:, :, WP - 3:WP], 0.0)
            nc.vector.memset(f_t[:, FR * WP:FALLOC], 0.0)
            if s == 0:
                nc.vector.memset(f_t[:, 0:3 * WP], 0.0)
            # ---- heads ----
            # stage1: 7 accumulating matmuls over dy -> Q[(c,dx), 512] PSUM
            # DVE copy -> SBUF, per-channel reshape-DMA into the block tile
            # Q''[c*42+jj, dx*512+u], ONE shear tensor_reduce over dx per
            # channel; d dumped fp32, ref channels cast to bf16 (halves D2H)
            import concourse.bass as _bass
            bf16 = mybir.dt.bfloat16
            j0 = 0
            for blkn in BLKS:
                qq = sbq.tile([96, 7 * 512], f32, tag="qq")
                for jj in range(blkn):
                    p0 = (j0 + jj) * CH
                    q = psq.tile([21, 512], f32, tag="q")
                    for dy in range(7):
                        nc.tensor.matmul(
                            out=q[:],
                            lhsT=wpk[0:C, C + dy * 21:C + (dy + 1) * 21],
                            rhs=f_t[:, p0 + dy * WP:p0 + dy * WP + 512],
                            start=(dy == 0), stop=(dy == 6))
                    qs = sbs.tile([21, 512], f32, tag="qs")
                    nc.vector.tensor_copy(qs[:], q[:])
                    for c in range(3):
                        nc.sync.dma_start(
                            out=qq[32 * c + jj:32 * c + jj + 1, :],
                            in_=qs[7 * c:7 * c + 7, :])
                red = sbr.tile([96, CH], f32, tag="red")
                for c in range(3):
                    rv = qq[32 * c:32 * c + blkn, 0:3584]
                    shear = _bass.AP(rv.tensor, rv.offset,
                                     [list(rv.ap[0]), [1, CH], [513, 7]])
                    nc.vector.tensor_reduce(
                        out=red[32 * c:32 * c + blkn, :], in_=shear,
                        axis=mybir.AxisListType.X, op=mybir.AluOpType.add)
                rb = sbb.tile([64, CH], bf16, tag="rb")
                nc.vector.tensor_copy(rb[0:blkn, :], red[32:32 + blkn, :])
                nc.vector.tensor_copy(rb[32:32 + blkn, :],
                                      red[64:64 + blkn, :])
                seg0 = s * PLANE + j0 * CH
                u32 = mybir.dt.uint32
                # pack d chunk-rows into d_all: chunk k -> partition k%128,
                # col (k//128)*CH (split on the 128-partition wrap)
                k0 = s * NCH + j0
                n1 = min(blkn, 128 - (k0 % 128))
                nc.sync.dma_start(
                    out=dall[k0 % 128:k0 % 128 + n1,
                             (k0 // 128) * CH:(k0 // 128) * CH + CH],
                    in_=red[0:n1, :])
                if blkn > n1:
                    k1 = k0 + n1
                    nc.sync.dma_start(
                        out=dall[k1 % 128:k1 % 128 + blkn - n1,
                                 (k1 // 128) * CH:(k1 // 128) * CH + CH],
                        in_=red[n1:blkn, :])
                for c in range(2):
                    w0 = (c * nss * PLANE + seg0) // 2
                    nc.sync.dma_start(
                        out=out_d[w0:w0 + blkn * CH // 2],
                        in_=rb[32 * c:32 * c + blkn, :].bitcast(u32))
                j0 += blkn

        # ---- per-partition top-R of d via max_with_indices/match_replace ----
        u32 = mybir.dt.uint32
        tv = sbk.tile([128, NTOPR], f32, tag="tv")
        ti = sbk.tile([128, NTOPR], u32, tag="ti")
        dv = sbk.tile([128, 8], f32, tag="dv")
        for r in range(NTOPR // 8):
            nc.vector.max_with_indices(tv[:, 8 * r:8 * r + 8],
                                       ti[:, 8 * r:8 * r + 8], dall[:])
            nc.vector.match_replace(dall[:], tv[:, 8 * r:8 * r + 8],
                                    dall[:], -1e30)
        nc.vector.max(dv[:], dall[:])   # first dropped value (certificate)
        npl = nss * PLANE
        nc.sync.dma_start(out=out_d[npl:npl + 128 * NTOPR],
                          in_=tv[:].bitcast(u32))
        nc.sync.dma_start(out=out_d[npl + 128 * NTOPR:npl + 256 * NTOPR],
                          in_=ti[:])
        nc.sync.dma_start(out=out_d[npl + 256 * NTOPR:npl + 256 * NTOPR + 1024],
                          in_=dv[:].bitcast(u32))


def _decode_out(raw, nss=NSS):
    """Packed u32 [nss*PLANE + TOPW]: 2 bf16 ref planes + topk block.

    Returns (ref [2, rows, WP] fp32, tops_v [128,R], tops_i [128,R],
    dropped_max scalar)."""
    import ml_dtypes
    npl = nss * PLANE
    r4 = raw[:npl].view(ml_dtypes.bfloat16).astype(np.float32)
    r4 = r4.reshape(2, nss, PLANE)[:, :, :NPOS]
    ref = r4.reshape(2, nss * SS, WP)
    tk = raw[npl:]
    tv = tk[:128 * NTOPR].view(np.float32).reshape(128, NTOPR)
    ti = tk[128 * NTOPR:256 * NTOPR].view(np.uint32).reshape(128, NTOPR)
    dv = tk[256 * NTOPR:256 * NTOPR + 1024].view(np.float32).reshape(128, 8)
    return ref, tv, ti, float(dv[:, 0].max())


def _candidates(tv, ti, h):
    """Decode shipped per-partition top-R into (linear image idx, d value).

    Slot -> chunk k = part + 128*(idx//CH), u = idx%CH; local position
    p = (k%NCH)*CH + u in superslab k//NCH; filter tail pads and the
    q>=512 garbage columns; unflip rows for bottom-half cores."""
    part = np.repeat(np.arange(128), NTOPR)
    val = tv.ravel()
    idx = ti.ravel().astype(np.int64)
    k = part + 128 * (idx // CH)
    u = idx % CH
    s, j = k // NCH, k % NCH
    p = j * CH + u
    ok = (k < NSS * NCH) & (p < NPOS) & (p % WP < W) & (val > -1e29)
    val, s, p = val[ok], s[ok], p[ok]
    yh = s * SS + p // WP
    x = p % WP
    yimg = yh if h == 0 else (H - 1) - yh
    return yimg * W + x, val


def _host_d_map(x, w_bb, b_bb, w_score):
    """Fallback: exact d map on host (only if the topk certificate fails)."""
    from numpy.lib.stride_tricks import sliding_window_view
    w_d = (w_score[1] - w_score[0]).astype(np.float32)
    d = np.zeros((x.shape[0], H, W), np.float32)
    for b in range(x.shape[0]):
        xp = np.zeros((C_IN, H + 2, W + 2), np.float32)
        xp[:, 1:1 + H, 1:1 + W] = x[b]
        f = np.zeros((C, H, W), np.float32)
        wf = w_bb.reshape(C, C_IN, 9)
        for t in range(9):
            dy, dx = t // 3, t % 3
            f += np.tensordot(wf[:, :, t],
                              xp[:, dy:dy + H, dx:dx + W], 1)
        f = np.maximum(f + b_bb[:, None, None], 0.0).astype(np.float32)
        fp = np.zeros((C, H + 6, W + 6), np.float32)
        fp[:, 3:3 + H, 3:3 + W] = f
        for t in range(49):
            dy, dx = t // 7, t % 7
            d[b] += np.tensordot(w_d[:, dy, dx],
                                 fp[:, dy:dy + H, dx:dx + W], 1)
    return d


def _canvases(x):
    """Per-core zero-padded canvases [3, 264, 520]; core = 2*b + h.

    h=1 cores get a vertically flipped image (and row-flipped weights)
    so the out-of-image boundary is always at local canvas top."""
    cans = []
    for b in range(B):
        for h in range(2):
            xb = x[b] if h == 0 else x[b, :, ::-1, :]
            can = np.zeros((C_IN, CANH, CANW), np.float32)
            # local image row of canvas row i is i - 4; rows <0 are zero
            can[:, 4:4 + 260, 4:4 + W] = xb[:, 0:260, :]
            cans.append(can)
    return cans


def _detection_heads(x, top_idx, w_bb, b_bb, w_loc, w_fourier, b_loc, b_fourier):
    """loc/fourier head values at detections, recomputed from x patches."""
    w27r = np.ascontiguousarray(
        w_bb.transpose(1, 2, 3, 0).reshape(27, C))     # [(cin,dy,dx), cout]
    w22 = np.concatenate([w_loc, w_fourier], 0)        # [22,C,7,7]
    w22f = w22.reshape(22, C * 49)
    b22 = np.concatenate([b_loc, b_fourier], 0)
    xp = np.zeros((B, C_IN, H + 8, W + 8), np.float32)
    xp[:, :, 4:4 + H, 4:4 + W] = x
    head22 = np.zeros((B, N_DET, 22), np.float32)
    for b in range(B):
        py = top_idx[b] // W
        px = top_idx[b] % W
        # 9x9 x patches around each detection -> [N,3,9,9]
        ys = (py[:, None] + np.arange(9)[None, :])     # padded rows
        xs = (px[:, None] + np.arange(9)[None, :])
        x9 = xp[b][:, ys[:, :, None], xs[:, None, :]]  # [3,N,9,9]
        x9 = x9.transpose(1, 0, 2, 3)                  # [N,3,9,9]
        # im2col for the 7x7 window of f: [N, 27, 49]
        sw = np.lib.stride_tricks.sliding_window_view(x9, (3, 3), axis=(2, 3))
        # sw [N,3,7,7,3,3] -> [N, (cin,ky,kx), (u,v)]
        p27 = sw.transpose(0, 1, 4, 5, 2, 3).reshape(N_DET, 27, 49)
        fp = np.einsum("kc,nkp->ncp", w27r, p27.astype(np.float32),
                       dtype=np.float32)
        fp = np.maximum(fp + b_bb[None, :, None], 0.0).astype(np.float32)
        head22[b] = fp.reshape(N_DET, C * 49) @ w22f.T + b22[None, :]
    return head22


def kernel(x, w_bb, b_bb, w_score, b_score, w_loc, b_loc,
           w_fourier, b_fourier, w_ref, b_ref):
    x = np.asarray(x, np.float32)
    w_bb = np.asarray(w_bb, np.float32)
    b_bb = np.asarray(b_bb, np.float32)
    w_score = np.asarray(w_score, np.float32)
    b_score = np.asarray(b_score, np.float32)
    w_loc = np.asarray(w_loc, np.float32)
    b_loc = np.asarray(b_loc, np.float32)
    w_fourier = np.asarray(w_fourier, np.float32)
    b_fourier = np.asarray(b_fourier, np.float32)
    w_ref = np.asarray(w_ref, np.float32)
    b_ref = np.asarray(b_ref, np.float32)

    # ---- weights prep (h=1 cores get row-flipped kernels) ----
    w_d = (w_score[1] - w_score[0]).astype(np.float32)          # [C,7,7]
    whead = np.stack([w_d, w_ref[0], w_ref[1]], 0)              # [3,C,7,7]
    w27_h, wh_h = [], []
    for h in range(2):
        wb = w_bb if h == 0 else w_bb[:, :, ::-1, :]
        wh = whead if h == 0 else whead[:, :, ::-1, :]
        w27_h.append(np.ascontiguousarray(
            wb.transpose(2, 3, 1, 0).reshape(27, C)))  # [(dy2,dx2,cin), cout]
        wh_h.append(np.ascontiguousarray(
            wh.transpose(1, 2, 0, 3).reshape(C, 147))) # [cin, (dy,c,dx)]

    cans = _canvases(x)

    # ---- device run ----
    global _CACHED_NC, _RUNNER, LAST_EXEC_NS, LAST_DEVICE_S
    in_maps = []
    for core in range(8):
        xin = np.empty(XINLEN, np.float32)
        xin[0:XCN] = cans[core].ravel()
        xin[XCN:XCN + 27 * C] = w27_h[core % 2].ravel()
        xin[XCN + 27 * C:XCN + 27 * C + C * 147] = wh_h[core % 2].ravel()
        xin[XINLEN - C:XINLEN] = b_bb
        in_maps.append({"xin": xin})
    import time as _time
    if _RUNNER is None:
        # first call: compile + run via run_bass_kernel_spmd, then build
        # and warm the cached jitted runner for steady-state calls
        from concourse.bass_utils import run_bass_kernel_spmd
        _CACHED_NC = _build_device_program()
        _t0 = _time.time()
        res = run_bass_kernel_spmd(_CACHED_NC, in_maps,
                                   core_ids=list(range(8)))
        LAST_DEVICE_S = _time.time() - _t0
        LAST_EXEC_NS = res.exec_time_ns
        _RUNNER = _make_runner(_CACHED_NC)
        results = _RUNNER(in_maps)   # warm trace/compile cache
    else:
        _t0 = _time.time()
        results = _RUNNER(in_maps)
        LAST_DEVICE_S = _time.time() - _t0
        LAST_EXEC_NS = None

    # ---- host: assemble ref maps + top-k candidates ----
    ref_map = np.zeros((B, 2, H, W), np.float32)
    cand = [[] for _ in range(B)]
    dropped = np.full(B, -np.inf, np.float32)
    for core in range(8):
        b, h = core // 2, core % 2
        ref, tv, ti, dmax = _decode_out(results[core]["out"])
        refm = ref[:, :, :W]
        if h == 1:
            refm = refm[:, ::-1, :]   # unflip: local row Y is image row 511-Y
        sl = slice(h * HALF, (h + 1) * HALF)
        ref_map[b, 0, sl] = refm[0]
        ref_map[b, 1, sl] = refm[1]
        cand[b].append(_candidates(tv, ti, h))
        dropped[b] = max(dropped[b], dmax)
    ref_map = (MARGIN * np.tanh(ref_map + b_ref[None, :, None, None])).astype(np.float32)
    bd = np.float32(b_score[1] - b_score[0])

    def _fg(d):
        pos = d >= 0
        e = np.exp(np.where(pos, -d, d).astype(np.float32)).astype(np.float32)
        return np.where(pos, (np.float32(1.0) / (np.float32(1.0) + e)),
                        (e / (np.float32(1.0) + e))).astype(np.float32)

    # ---- top-k by softmax-foreground ordering (matches jax softmax+top_k)
    # from device-shipped per-partition top-R candidates; a strict-dominance
    # certificate guards exactness, with a host conv fallback ----
    top_idx = np.zeros((B, N_DET), np.int32)
    for b in range(B):
        lin = np.concatenate([c[0] for c in cand[b]])
        val = np.concatenate([c[1] for c in cand[b]]).astype(np.float32)
        ok = len(lin) >= N_DET
        if ok:
            fg = _fg(val + bd)
            order = np.lexsort((lin, -fg))[:N_DET]
            ok = val[order[-1]] > dropped[b]
        if ok:
            top_idx[b] = lin[order].astype(np.int32)
        else:   # certificate failed: exact host recomputation of d
            d_full = (_host_d_map(x[b:b + 1], w_bb, b_bb,
                                  w_score)[0].reshape(H * W) + bd)
            fg = _fg(d_full)
            top_idx[b] = np.argsort(-fg, kind="stable")[:N_DET].astype(np.int32)

    # ---- loc/fourier head at detections ----
    head22 = _detection_heads(x, top_idx, w_bb, b_bb, w_loc, w_fourier,
                              b_loc, b_fourier)
    px = (top_idx % W).astype(np.float32)
    py = (top_idx // W).astype(np.float32)
    loc = head22[..., 0:2]
    coef = head22[..., 2:22].reshape(B, N_DET, ORDER, 4)
    cx = (px + loc[..., 0]).astype(np.float32)
    cy = (py + loc[..., 1]).astype(np.float32)

    # ---- fourier contour synthesis ----
    t = np.arange(SAMPLES, dtype=np.float32) / np.float32(SAMPLES)
    kk = np.arange(1, ORDER + 1, dtype=np.float32)
    ang = (np.float32(2.0 * np.pi) * kk[:, None] * t[None, :]).astype(np.float32)
    cos_a = np.cos(ang).astype(np.float32)
    sin_a = np.sin(ang).astype(np.float32)
    xs = (np.einsum("bno,os->bns", coef[..., 0], cos_a, dtype=np.float32)
          + np.einsum("bno,os->bns", coef[..., 1], sin_a, dtype=np.float32)
          + cx[..., None]).astype(np.float32)
    ys = (np.einsum("bno,os->bns", coef[..., 2], cos_a, dtype=np.float32)
          + np.einsum("bno,os->bns", coef[..., 3], sin_a, dtype=np.float32)
          + cy[..., None]).astype(np.float32)
    det = np.stack([xs, ys], -1)

    # ---- refinement iterations ----
    ref_flat = ref_map.reshape(B, 2, H * W)
    for _ in range(ITERS):
        deti = np.round(det)
        xc = np.clip(deti[..., 0], 0, W - 1)
        yc = np.clip(deti[..., 1], 0, H - 1)
        lin = (yc.astype(np.int32) * W + xc.astype(np.int32)).reshape(B, N_DET * SAMPLES)
        rx = np.take_along_axis(ref_flat[:, 0], lin, 1).reshape(B, N_DET, SAMPLES)
        ry = np.take_along_axis(ref_flat[:, 1], lin, 1).reshape(B, N_DET, SAMPLES)
        det = np.stack([(xc + rx).astype(np.float32),
                        (yc + ry).astype(np.float32)], -1)
    return det.astype(np.float32)


# revision 47
# speedup vs baseline: 1.0162x; 1.0162x over previous
"""nn_CPN_67740224192953 kernel: detection pipeline on 8 trn2 cores.

Device (8 cores, 2 per image = half-image each; bottom halves are
processed vertically flipped with row-flipped weights so the zero-pad
boundary is uniform across cores), fp32 compute, ~2ms NEFF:
  - im2col of the 3-channel x canvas via 9 strided DMAs (27-row rhs)
  - backbone 3x3 conv as K=27 matmuls + relu(+bias) on ACT
    -> f [64, 70*518] SBUF per 64-row superslab
  - head convs for the 3 channels needing full-res maps
    (d = score1-score0, ref_x, ref_y) factored over dy:
      stage1: 7 accumulating matmuls (K=64, M=21=(c,dx)) into PSUM
      stage2: PSUM->SBUF copy, reshape-DMA into a (c*32+jj)-partition
      block tile Q''[.., dx*512+u], one shear tensor_reduce over dx
  - ref channels dumped bf16; the d plane never leaves the device:
    per-partition top-32 candidates (max_with_indices/match_replace,
    4 rounds) + a first-dropped certificate value are shipped instead
    (~0.57MB/core out, ~1.7MB/core in through the axon tunnel)
The device program and a jitted SPMD runner (device-resident zero
output operands, no per-call retrace) are cached at module level.
Host: bias/tanh/margin, exact top-k from the shipped candidates (a
strict-dominance certificate guards exactness; host conv fallback if
it ever fails), loc/fourier head at the 512 detections (recomputed
from x patches), fourier contour synthesis, 4 refinement gather
iterations (mirrors reference).
"""

import numpy as np

LAST_EXEC_NS = None
LAST_DEVICE_S = None

B, C_IN, H, W = 4, 3, 512, 512
C = 64
ORDER = 5
SAMPLES = 32
N_DET = 512
ITERS = 4
MARGIN = 3.0
K7 = 7

HALF = 256            # output rows per core
WP = 518              # f-space width (x_img = q-3, q in [0,518))
CANH, CANW = 264, 520 # x canvas per core (y_img = i-4+256h, x_img = j-4)
SS = 64               # output rows per superslab
NSS = HALF // SS      # 4 superslabs
FR = SS + 6           # f rows per superslab
BROWS = 7             # f rows per backbone band
NBAND = (FR + BROWS - 1) // BROWS   # 10 bands
BPOS = BROWS * WP     # f positions per full band
NPOS = SS * WP        # 33152 out positions per superslab
CH = 506              # out positions per head chunk (512 - 6 halo)
NCH = (NPOS + CH - 1) // CH   # 66 chunks (65*506 + 262)
BLKS = (32, 32, 2)    # chunks per stage2 block (c*32+jj, quadrant-aligned)
FALLOC = FR * WP + 252        # f slack: last chunk reads to 36510
PLANE = NCH * CH              # 33396 positions per superslab incl. tail pad
NTOPR = 32            # top-R values extracted per partition (4 rounds of 8)
DALLW = 3 * CH        # d_all free width (1518; 264 chunk rows in 128x3 slots)
TOPW = 128 * NTOPR * 2 + 128 * 8      # u32 words: values + indices + dropped
XCN = C_IN * CANH * CANW      # canvas floats
XINLEN = XCN + 27 * C + C * 147 + C   # packed input floats per core

_CACHED_NC = None
_RUNNER = None


def _make_runner(nc):
    """Jitted SPMD runner built once and reused across kernel() calls
    (mirrors bass2jax.run_bass_via_pjrt's multi-core path; caching the
    traced executable removes per-call retrace/lowering overhead)."""
    import jax
    import numpy as np
    from jax.experimental.shard_map import shard_map
    from jax.sharding import Mesh, PartitionSpec
    from concourse import bass2jax, mybir

    bass2jax.install_neuronx_cc_hook()
    n_cores = 8
    partition_name = (nc.partition_id_tensor.name
                      if nc.partition_id_tensor else None)
    in_names, out_names, out_avals = [], [], []
    for alloc in nc.m.functions[0].allocations:
        if not isinstance(alloc, mybir.MemoryLocationSet):
            continue
        name = alloc.memorylocations[0].name
        if alloc.kind == "ExternalInput":
            if name != partition_name:
                in_names.append(name)
        elif alloc.kind == "ExternalOutput":
            shape = tuple(alloc.tensor_shape)
            dtype = mybir.dt.np(alloc.dtype)
            out_names.append(name)
            out_avals.append(jax.core.ShapedArray(shape, dtype))
    n_params = len(in_names)
    all_names = in_names + out_names
    if partition_name is not None:
        all_names = all_names + [partition_name]
    donate = tuple(range(n_params, n_params + len(out_names)))

    def _body(*args):
        operands = list(args)
        if partition_name is not None:
            operands.append(bass2jax.partition_id_tensor())
        outs = bass2jax._bass_exec_p.bind(
            *operands,
            out_avals=tuple(out_avals),
            in_names=tuple(all_names),
            out_names=tuple(out_names),
            lowering_input_output_aliases=(),
            sim_require_finite=True,
            sim_require_nnan=True,
            nc=nc,
        )
        return tuple(outs)

    devices = jax.devices()[:n_cores]
    mesh = Mesh(np.asarray(devices), ("core",))
    nargs = n_params + len(out_names)
    sharded = jax.jit(
        shard_map(_body, mesh=mesh,
                  in_specs=(PartitionSpec("core"),) * nargs,
                  out_specs=(PartitionSpec("core"),) * len(out_names),
                  check_rep=False),
        keep_unused=True)

    # outputs are fully written by the NEFF, so the zero "output operand"
    # buffers never need refreshing: keep them device-resident across calls
    from jax.sharding import NamedSharding
    shard8 = NamedSharding(mesh, PartitionSpec("core"))
    zeros_dev = [jax.device_put(
        np.zeros((n_cores * a.shape[0], *a.shape[1:]), a.dtype), shard8)
        for a in out_avals]

    def run(in_maps):
        per_core = [[np.asarray(m[name]) for name in in_names] for m in in_maps]
        concat_in = [np.concatenate([per_core[c][i] for c in range(n_cores)], 0)
                     for i in range(n_params)]
        outs = sharded(*concat_in, *zeros_dev)
        return [{name: np.asarray(outs[i]).reshape(n_cores, *out_avals[i].shape)[c]
                 for i, name in enumerate(out_names)}
                for c in range(n_cores)]

    return run


def _build_device_program(num_devices=8, nss=NSS):
    import concourse.bacc as bacc
    import concourse.mybir as mybir

    nc = bacc.Bacc("TRN2", target_bir_lowering=False, num_devices=num_devices)
    f32 = mybir.dt.float32
    xin_d = nc.dram_tensor("xin", [XINLEN], f32, kind="ExternalInput")
    xc_d = xin_d[0:XCN].rearrange("(c h w) -> c h w", c=C_IN, h=CANH)
    w27_d = xin_d[XCN:XCN + 27 * C].rearrange("(a b) -> a b", a=27)
    wh_d = xin_d[XCN + 27 * C:XCN + 27 * C + C * 147].rearrange(
        "(a b) -> a b", a=C)
    bbb_d = xin_d[XINLEN - C:XINLEN].rearrange("(a b) -> a b", a=C)
    out_d = nc.dram_tensor("out", [nss * PLANE + TOPW], mybir.dt.uint32,
                           kind="ExternalOutput")
    _device_body(nc, xc_d, w27_d, wh_d, bbb_d, out_d, nss)
    nc.finalize()
    return nc


def _device_body(nc, xc_d, w27_d, wh_d, bbb_d, out_d, nss=NSS):
    import concourse.mybir as mybir
    from concourse.tile import TileContext

    f32 = mybir.dt.float32
    with (
        TileContext(nc) as tc,
        tc.tile_pool(name="wpool", bufs=1) as wpool,
        tc.tile_pool(name="sbi", bufs=2) as sbi,
        tc.tile_pool(name="sbf", bufs=1) as sbf,
        tc.tile_pool(name="sbq", bufs=1) as sbq,
        tc.tile_pool(name="sbs", bufs=1) as sbs,
        tc.tile_pool(name="sbr", bufs=1) as sbr,
        tc.tile_pool(name="sbb", bufs=1) as sbb,
        tc.tile_pool(name="sbk", bufs=1) as sbk,
        tc.tile_pool(name="ps", bufs=2, space="PSUM") as ps,
        tc.tile_pool(name="psq", bufs=4, space="PSUM") as psq,
    ):
        # weights: DMA into one packed staging tile, DVE-copy to a packed
        # working tile (cheap matmul weight deps); layout: cols [0:64) w27
        # (parts 0:27), [64:211) wh (parts 0:64), [211:212) bbb
        wst = wpool.tile([C, 212], f32, tag="wst")
        nc.sync.dma_start(out=wst[0:27, 0:C], in_=w27_d[:, :])
        nc.sync.dma_start(out=wst[0:C, C:C + 147], in_=wh_d[:, :])
        nc.sync.dma_start(out=wst[0:C, 211:212], in_=bbb_d[:, :])
        wpk = wpool.tile([C, 212], f32, tag="wpk")
        nc.vector.tensor_copy(wpk[0:27, 0:C], wst[0:27, 0:C])
        nc.vector.tensor_copy(wpk[0:C, C:C + 147], wst[0:C, C:C + 147])
        nc.vector.tensor_copy(wpk[0:C, 211:212], wst[0:C, 211:212])
        w27_t = wpk[0:27, 0:C]
        wh_t = wpk[0:C, C:C + 147]
        bbb_t = wpk[0:C, 211:212]

        dall = sbk.tile([128, DALLW], f32, tag="dall")
        nc.vector.memset(dall[:], -1e30)

        for s in range(nss):
            f_t = sbf.tile([C, FALLOC], f32, tag="f")
            # ---- backbone: f = relu(w27.T @ im2col(x) + b) ----
            for band in range(NBAND):
                brows = min(BROWS, FR - band * BROWS)
                bpos = brows * WP
                r0 = s * SS + band * BROWS   # global f row of band start
                imc = sbi.tile([27, BPOS], f32, tag="imc")
                for dy2 in range(3):
                    for dx2 in range(3):
                        k = dy2 * 3 + dx2
                        nc.sync.dma_start(
                            out=imc[3 * k:3 * k + 3, :bpos],
                            in_=xc_d[:, r0 + dy2:r0 + dy2 + brows,
                                     dx2:dx2 + WP])
                boff = band * BPOS
                for c0 in range(0, bpos, 512):
                    n = min(512, bpos - c0)
                    pbb = ps.tile([C, 512], f32, tag="pbb")
                    nc.tensor.matmul(out=pbb[:, :n], lhsT=w27_t,
                                     rhs=imc[:, c0:c0 + n],
                                     start=True, stop=True)
                    nc.scalar.activation(
                        f_t[:, boff + c0:boff + c0 + n], pbb[:, :n],
                        mybir.ActivationFunctionType.Relu,
                        bias=bbb_t)
            # f must be zero where the head conv expects zero padding:
            # cols q<3 / q>=515 (x_img outside) on every superslab, and
            # rows rf<3 of superslab 0 (y outside; bottom cores get a
            # vertically flipped canvas so the edge is always local-top).
            f3d = f_t[:, 0:FR * WP].rearrange("p (r q) -> p r q", q=WP)
            nc.vector.memset(f3d[:, :, 0:3], 0.0)
            nc.vector.memset(f3d[:, :, WP - 3:WP], 0.0)
            nc.vector.memset(f_t[:, FR * WP:FALLOC], 0.0)
            if s == 0:
                nc.vector.memset(f_t[:, 0:3 * WP], 0.0)
            # ---- heads ----
            # stage1: 7 accumulating matmuls over dy -> Q[(c,dx), 512] PSUM
            # DVE copy -> SBUF, per-channel reshape-DMA into the block tile
            # Q''[c*42+jj, dx*512+u], ONE shear tensor_reduce over dx per
            # channel; d dumped fp32, ref channels cast to bf16 (halves D2H)
            import concourse.bass as _bass
            bf16 = mybir.dt.bfloat16
            j0 = 0
            for blkn in BLKS:
                qq = sbq.tile([96, 7 * 512], f32, tag="qq")
                for jj in range(blkn):
                    p0 = (j0 + jj) * CH
                    q = psq.tile([21, 512], f32, tag="q")
                    for dy in range(7):
                        nc.tensor.matmul(
                            out=q[:],
                            lhsT=wpk[0:C, C + dy * 21:C + (dy + 1) * 21],
                            rhs=f_t[:, p0 + dy * WP:p0 + dy * WP + 512],
                            start=(dy == 0), stop=(dy == 6))
                    qs = sbs.tile([21, 512], f32, tag="qs")
                    nc.vector.tensor_copy(qs[:], q[:])
                    for c in range(3):
                        nc.sync.dma_start(
                            out=qq[32 * c + jj:32 * c + jj + 1, :],
                            in_=qs[7 * c:7 * c + 7, :])
                red = sbr.tile([96, CH], f32, tag="red")
                for c in range(3):
                    rv = qq[32 * c:32 * c + blkn, 0:3584]
                    shear = _bass.AP(rv.tensor, rv.offset,
                                     [list(rv.ap[0]), [1, CH], [513, 7]])
                    nc.vector.tensor_reduce(
                        out=red[32 * c:32 * c + blkn, :], in_=shear,
                        axis=mybir.AxisListType.X, op=mybir.AluOpType.add)
                rb = sbb.tile([64, CH], bf16, tag="rb")
                nc.vector.tensor_copy(rb[0:blkn, :], red[32:32 + blkn, :])
                nc.vector.tensor_copy(rb[32:32 + blkn, :],
                                      red[64:64 + blkn, :])
                seg0 = s * PLANE + j0 * CH
                u32 = mybir.dt.uint32
                # pack d chunk-rows into d_all: chunk k -> partition k%128,
                # col (k//128)*CH (split on the 128-partition wrap)
                k0 = s * NCH + j0
                n1 = min(blkn, 128 - (k0 % 128))
                nc.sync.dma_start(
                    out=dall[k0 % 128:k0 % 128 + n1,
                             (k0 // 128) * CH:(k0 // 128) * CH + CH],
                    in_=red[0:n1, :])
                if blkn > n1:
                    k1 = k0 + n1
                    nc.sync.dma_start(
                        out=dall[k1 % 128:k1 % 128 + blkn - n1,
                                 (k1 // 128) * CH:(k1 // 128) * CH + CH],
                        in_=red[n1:blkn, :])
                for c in range(2):
                    w0 = (c * nss * PLANE + seg0) // 2
                    nc.sync.dma_start(
                        out=out_d[w0:w0 + blkn * CH // 2],
                        in_=rb[32 * c:32 * c + blkn, :].bitcast(u32))
                j0 += blkn

        # ---- per-partition top-R of d via max_with_indices/match_replace ----
        u32 = mybir.dt.uint32
        tv = sbk.tile([128, NTOPR], f32, tag="tv")
        ti = sbk.tile([128, NTOPR], u32, tag="ti")
        dv = sbk.tile([128, 8], f32, tag="dv")
        for r in range(NTOPR // 8):
            nc.vector.max_with_indices(tv[:, 8 * r:8 * r + 8],
                                       ti[:, 8 * r:8 * r + 8], dall[:])
            nc.vector.match_replace(dall[:], tv[:, 8 * r:8 * r + 8],
                                    dall[:], -1e30)
        nc.vector.max(dv[:], dall[:])   # first dropped value (certificate)
        npl = nss * PLANE
        nc.sync.dma_start(out=out_d[npl:npl + 128 * NTOPR],
                          in_=tv[:].bitcast(u32))
        nc.sync.dma_start(out=out_d[npl + 128 * NTOPR:npl + 256 * NTOPR],
                          in_=ti[:])
        nc.sync.dma_start(out=out_d[npl + 256 * NTOPR:npl + 256 * NTOPR + 1024],
                          in_=dv[:].bitcast(u32))


def _decode_out(raw, nss=NSS):
    """Packed u32 [nss*PLANE + TOPW]: 2 bf16 ref planes + topk block.

    Returns (ref [2, rows, WP] fp32, tops_v [128,R], tops_i [128,R],
    dropped_max scalar)."""
    import ml_dtypes
    npl = nss * PLANE
    r4 = raw[:npl].view(ml_dtypes.bfloat16).astype(np.float32)
    r4 = r4.reshape(2, nss, PLANE)[:, :, :NPOS]
    ref = r4.reshape(2, nss * SS, WP)
    tk = raw[npl:]
    tv = tk[:128 * NTOPR].view(np.float32).reshape(128, NTOPR)
    ti = tk[128 * NTOPR:256 * NTOPR].view(np.uint32).reshape(128, NTOPR)
    dv = tk[256 * NTOPR:256 * NTOPR + 1024].view(np.float32).reshape(128, 8)
    return ref, tv, ti, float(dv[:, 0].max())


def _candidates(tv, ti, h):
    """Decode shipped per-partition top-R into (linear image idx, d value).

    Slot -> chunk k = part + 128*(idx//CH), u = idx%CH; local position
    p = (k%NCH)*CH + u in superslab k//NCH; filter tail pads and the
    q>=512 garbage columns; unflip rows for bottom-half cores."""
    part = np.repeat(np.arange(128), NTOPR)
    val = tv.ravel()
    idx = ti.ravel().astype(np.int64)
    k = part + 128 * (idx // CH)
    u = idx % CH
    s, j = k // NCH, k % NCH
    p = j * CH + u
    ok = (k < NSS * NCH) & (p < NPOS) & (p % WP < W) & (val > -1e29)
    val, s, p = val[ok], s[ok], p[ok]
    yh = s * SS + p // WP
    x = p % WP
    yimg = yh if h == 0 else (H - 1) - yh
    return yimg * W + x, val


def _host_d_map(x, w_bb, b_bb, w_score):
    """Fallback: exact d map on host (only if the topk certificate fails)."""
    from numpy.lib.stride_tricks import sliding_window_view
    w_d = (w_score[1] - w_score[0]).astype(np.float32)
    d = np.zeros((x.shape[0], H, W), np.float32)
    for b in range(x.shape[0]):
        xp = np.zeros((C_IN, H + 2, W + 2), np.float32)
        xp[:, 1:1 + H, 1:1 + W] = x[b]
        f = np.zeros((C, H, W), np.float32)
        wf = w_bb.reshape(C, C_IN, 9)
        for t in range(9):
            dy, dx = t // 3, t % 3
            f += np.tensordot(wf[:, :, t],
                              xp[:, dy:dy + H, dx:dx + W], 1)
        f = np.maximum(f + b_bb[:, None, None], 0.0).astype(np.float32)
        fp = np.zeros((C, H + 6, W + 6), np.float32)
        fp[:, 3:3 + H, 3:3 + W] = f
        for t in range(49):
            dy, dx = t // 7, t % 7
            d[b] += np.tensordot(w_d[:, dy, dx],
                                 fp[:, dy:dy + H, dx:dx + W], 1)
    return d


def _canvases(x):
    """Per-core zero-padded canvases [3, 264, 520]; core = 2*b + h.

    h=1 cores get a vertically flipped image (and row-flipped weights)
    so the out-of-image boundary is always at local canvas top."""
    cans = []
    for b in range(B):
        for h in range(2):
            xb = x[b] if h == 0 else x[b, :, ::-1, :]
            can = np.zeros((C_IN, CANH, CANW), np.float32)
            # local image row of canvas row i is i - 4; rows <0 are zero
            can[:, 4:4 + 260, 4:4 + W] = xb[:, 0:260, :]
            cans.append(can)
    return cans


def _detection_heads(x, top_idx, w_bb, b_bb, w_loc, w_fourier, b_loc, b_fourier):
    """loc/fourier head values at detections, recomputed from x patches."""
    w27r = np.ascontiguousarray(
        w_bb.transpose(1, 2, 3, 0).reshape(27, C))     # [(cin,dy,dx), cout]
    w22 = np.concatenate([w_loc, w_fourier], 0)        # [22,C,7,7]
    w22f = w22.reshape(22, C * 49)
    b22 = np.concatenate([b_loc, b_fourier], 0)
    xp = np.zeros((B, C_IN, H + 8, W + 8), np.float32)
    xp[:, :, 4:4 + H, 4:4 + W] = x
    head22 = np.zeros((B, N_DET, 22), np.float32)
    for b in range(B):
        py = top_idx[b] // W
        px = top_idx[b] % W
        # 9x9 x patches around each detection -> [N,3,9,9]
        ys = (py[:, None] + np.arange(9)[None, :])     # padded rows
        xs = (px[:, None] + np.arange(9)[None, :])
        x9 = xp[b][:, ys[:, :, None], xs[:, None, :]]  # [3,N,9,9]
        x9 = x9.transpose(1, 0, 2, 3)                  # [N,3,9,9]
        # im2col for the 7x7 window of f: [N, 27, 49]
        sw = np.lib.stride_tricks.sliding_window_view(x9, (3, 3), axis=(2, 3))
        # sw [N,3,7,7,3,3] -> [N, (cin,ky,kx), (u,v)]
        p27 = sw.transpose(0, 1, 4, 5, 2, 3).reshape(N_DET, 27, 49)
        fp = np.einsum("kc,nkp->ncp", w27r, p27.astype(np.float32),
                       dtype=np.float32)
        fp = np.maximum(fp + b_bb[None, :, None], 0.0).astype(np.float32)
        head22[b] = fp.reshape(N_DET, C * 49) @ w22f.T + b22[None, :]
    return head22


def kernel(x, w_bb, b_bb, w_score, b_score, w_loc, b_loc,
           w_fourier, b_fourier, w_ref, b_ref):
    x = np.asarray(x, np.float32)
    w_bb = np.asarray(w_bb, np.float32)
    b_bb = np.asarray(b_bb, np.float32)
    w_score = np.asarray(w_score, np.float32)
    b_score = np.asarray(b_score, np.float32)
    w_loc = np.asarray(w_loc, np.float32)
    b_loc = np.asarray(b_loc, np.float32)
    w_fourier = np.asarray(w_fourier, np.float32)
    b_fourier = np.asarray(b_fourier, np.float32)
    w_ref = np.asarray(w_ref, np.float32)
    b_ref = np.asarray(b_ref, np.float32)

    # ---- weights prep (h=1 cores get row-flipped kernels) ----
    w_d = (w_score[1] - w_score[0]).astype(np.float32)          # [C,7,7]
    whead = np.stack([w_d, w_ref[0], w_ref[1]], 0)              # [3,C,7,7]
    w27_h, wh_h = [], []
    for h in range(2):
        wb = w_bb if h == 0 else w_bb[:, :, ::-1, :]
        wh = whead if h == 0 else whead[:, :, ::-1, :]
        w27_h.append(np.ascontiguousarray(
            wb.transpose(2, 3, 1, 0).reshape(27, C)))  # [(dy2,dx2,cin), cout]
        wh_h.append(np.ascontiguousarray(
            wh.transpose(1, 2, 0, 3).reshape(C, 147))) # [cin, (dy,c,dx)]

    cans = _canvases(x)

    # ---- device run ----
    global _CACHED_NC, _RUNNER, LAST_EXEC_NS, LAST_DEVICE_S
    in_maps = []
    for core in range(8):
        xin = np.empty(XINLEN, np.float32)
        xin[0:XCN] = cans[core].ravel()
        xin[XCN:XCN + 27 * C] = w27_h[core % 2].ravel()
        xin[XCN + 27 * C:XCN + 27 * C + C * 147] = wh_h[core % 2].ravel()
        xin[XINLEN - C:XINLEN] = b_bb
        in_maps.append({"xin": xin})
    import time as _time
    if _RUNNER is None:
        # first call: compile + run via run_bass_kernel_spmd, then build
        # and warm the cached jitted runner for steady-state calls
        from concourse.bass_utils import run_bass_kernel_spmd
        _CACHED_NC = _build_device_program()
        _t0 = _time.time()
        res = run_bass_kernel_spmd(_CACHED_NC, in_maps,
                                   core_ids=list(range(8)))
        LAST_DEVICE_S = _time.time() - _t0
        LAST_EXEC_NS = res.exec_time_ns
        _RUNNER = _make_runner(_CACHED_NC)
        results = _RUNNER(in_maps)   # warm trace/compile cache
    else:
        _t0 = _time.time()
        results = _RUNNER(in_maps)
        LAST_DEVICE_S = _time.time() - _t0
        LAST_EXEC_NS = None

    # ---- host: assemble ref maps + top-k candidates ----
    ref_map = np.zeros((B, 2, H, W), np.float32)
    cand = [[] for _ in range(B)]
    dropped = np.full(B, -np.inf, np.float32)
    for core in range(8):
        b, h = core // 2, core % 2
        ref, tv, ti, dmax = _decode_out(results[core]["out"])
        refm = ref[:, :, :W]
        if h == 1:
            refm = refm[:, ::-1, :]   # unflip: local row Y is image row 511-Y
        sl = slice(h * HALF, (h + 1) * HALF)
        ref_map[b, 0, sl] = refm[0]
        ref_map[b, 1, sl] = refm[1]
        cand[b].append(_candidates(tv, ti, h))
        dropped[b] = max(dropped[b], dmax)
    ref_map = (MARGIN * np.tanh(ref_map + b_ref[None, :, None, None])).astype(np.float32)
    bd = np.float32(b_score[1] - b_score[0])

    def _fg(d):
        pos = d >= 0
        e = np.exp(np.where(pos, -d, d).astype(np.float32)).astype(np.float32)
        return np.where(pos, (np.float32(1.0) / (np.float32(1.0) + e)),
                        (e / (np.float32(1.0) + e))).astype(np.float32)

    # ---- top-k by softmax-foreground ordering (matches jax softmax+top_k)
    # from device-shipped per-partition top-R candidates; a strict-dominance
    # certificate guards exactness, with a host conv fallback ----
    top_idx = np.zeros((B, N_DET), np.int32)
    for b in range(B):
        lin = np.concatenate([c[0] for c in cand[b]])
        val = np.concatenate([c[1] for c in cand[b]]).astype(np.float32)
        ok = len(lin) >= N_DET
        if ok:
            fg = _fg(val + bd)
            order = np.lexsort((lin, -fg))[:N_DET]
            ok = val[order[-1]] > dropped[b]
        if ok:
            top_idx[b] = lin[order].astype(np.int32)
        else:   # certificate failed: exact host recomputation of d
            d_full = (_host_d_map(x[b:b + 1], w_bb, b_bb,
                                  w_score)[0].reshape(H * W) + bd)
            fg = _fg(d_full)
            top_idx[b] = np.argsort(-fg, kind="stable")[:N_DET].astype(np.int32)

    # ---- loc/fourier head at detections ----
    head22 = _detection_heads(x, top_idx, w_bb, b_bb, w_loc, w_fourier,
                              b_loc, b_fourier)
    px = (top_idx % W).astype(np.float32)
    py = (top_idx // W).astype(np.float32)
    loc = head22[..., 0:2]
    coef = head22[..., 2:22].reshape(B, N_DET, ORDER, 4)
    cx = (px + loc[..., 0]).astype(np.float32)
    cy = (py + loc[..., 1]).astype(np.float32)

    # ---- fourier contour synthesis ----
    t = np.arange(SAMPLES, dtype=np.float32) / np.float32(SAMPLES)
    kk = np.arange(1, ORDER + 1, dtype=np.float32)
    ang = (np.float32(2.0 * np.pi) * kk[:, None] * t[None, :]).astype(np.float32)
    cos_a = np.cos(ang).astype(np.float32)
    sin_a = np.sin(ang).astype(np.float32)
    xs = (np.einsum("bno,os->bns", coef[..., 0], cos_a, dtype=np.float32)
          + np.einsum("bno,os->bns", coef[..., 1], sin_a, dtype=np.float32)
          + cx[..., None]).astype(np.float32)
    ys = (np.einsum("bno,os->bns", coef[..., 2], cos_a, dtype=np.float32)
          + np.einsum("bno,os->bns", coef[..., 3], sin_a, dtype=np.float32)
          + cy[..., None]).astype(np.float32)
    det = np.stack([xs, ys], -1)

    # ---- refinement iterations ----
    ref_flat = ref_map.reshape(B, 2, H * W)
    for _ in range(ITERS):
        deti = np.round(det)
        xc = np.clip(deti[..., 0], 0, W - 1)
        yc = np.clip(deti[..., 1], 0, H - 1)
        lin = (yc.astype(np.int32) * W + xc.astype(np.int32)).reshape(B, N_DET * SAMPLES)
        rx = np.take_along_axis(ref_flat[:, 0], lin, 1).reshape(B, N_DET, SAMPLES)
        ry = np.take_along_axis(ref_flat[:, 1], lin, 1).reshape(B, N_DET, SAMPLES)
        det = np.stack([(xc + rx).astype(np.float32),
                        (yc + ry).astype(np.float32)], -1)
    return det.astype(np.float32)
